# revision 3
# baseline (speedup 1.0000x reference)
"""Trainium2 Bass kernel for nn_DynamicEmbedder (MoE-style routed embedding + projection).

Reference computation (fp32):
    is_high = node_ids < 100_000
    out[b]  = is_high ? emb_high_w[id] @ W_high.T + b_high
                      : emb_low_w[id - 100_000] @ W_low.T + b_low

Strategy (8 NeuronCores):
  * Host-side routing ("all-to-all, expert-parallel style" per the sharding
    hint): tokens are bucketed by the core that owns their embedding row.
    Tables are row-sharded 8 ways (high: 12500 rows/core, low: 112500).
    The low shard is further split into 4 sub-shards of 28125 rows so local
    row indices fit the int16 index format of the SWDGE dma_gather op.
  * Per core the token list is a "high" section plus 4 "low" subsections,
    each padded to a multiple of 128, so the device program is fully
    static: no on-device select/masking and no wasted dual-expert compute.
  * Device program per 128-token sub-tile:
       dma_gather rows         -> [128 tok, D] in SBUF   (SWDGE)
       PE transpose            -> [D, 128 tok] in PSUM
       DVE copy                -> SBUF
       PE matmul  X^T as lhsT  -> PSUM [128 tok, 128 feat]  (fp32)
       ACT copy                -> SBUF
       HWDGE DMA               -> DRAM output rows
  * Host inverse-permutes the concatenated per-core outputs and adds the
    (normally zero) biases.
"""

import math
import os
import sys

import numpy as np

for _p in ("/opt/trn_rl_repo", "/opt/pypackages"):
    if _p not in sys.path:
        sys.path.append(_p)

import concourse.bass as bass
import concourse.mybir as mybir
import concourse.tile as tile
from concourse import bacc
from concourse.bass_utils import run_bass_kernel_spmd

# Problem constants (hardcoded per the harness contract).
NUM_NODES = 1_000_000
NUM_HIGH = 100_000
NUM_LOW = NUM_NODES - NUM_HIGH
D_HIGH, D_LOW, D_OUT = 256, 64, 128
BATCH = 500_000
N_CORES = 8
NHI_SHARD = NUM_HIGH // N_CORES   # 12500 high rows per core (< int16 max)
NLO_SHARD = NUM_LOW // N_CORES    # 112500 low rows per core
N_LO_SUB = 4
NLO_SUBSHARD = NLO_SHARD // N_LO_SUB  # 28125 rows per low sub-shard (< int16 max)

P = 128                  # SBUF partitions / tokens per sub-tile
QUAD = 4                 # sub-tiles per PSUM-output tile (512 tokens)
HI_GSUB = 8              # sub-tiles per high gather group (1024 tokens)
LO_GSUB = 16             # sub-tiles per low gather group  (2048 tokens)

F32 = mybir.dt.float32
I16 = mybir.dt.int16


def _build_program(hi_tiles, lo_sub_tiles, nhi_rows, nlo_sub_rows,
                   d_hi, d_lo, d_out, n_reps=1, enable_asserts=False):
    """Build the (single-core, SPMD-replicated) Bass program.

    hi_tiles: number of 128-token sub-tiles in the high section.
    lo_sub_tiles: tuple of sub-tile counts, one per low sub-shard.
    """
    assert d_hi % P == 0 and d_out == P and d_lo <= P
    lo_sub_tiles = tuple(lo_sub_tiles)
    lo_tiles = sum(lo_sub_tiles)

    nc = bacc.Bacc(
        "TRN2",
        target_bir_lowering=False,
        debug=False,
        enable_asserts=enable_asserts,
        num_devices=N_CORES,
    )

    # int16 index tensors, [128, n_tokens/16]: token j of a section lives at
    # [j % 16, j // 16], replicated 8x down the partition dim (one copy per
    # GpSimd Q7 core).
    ids_hi = nc.dram_tensor("ids_hi", [P, hi_tiles * 8], I16, kind="ExternalInput")
    ids_lo = nc.dram_tensor("ids_lo", [P, lo_tiles * 8], I16, kind="ExternalInput")
    emb_hi = nc.dram_tensor("emb_hi", [nhi_rows, d_hi], F32, kind="ExternalInput")
    emb_lo = nc.dram_tensor("emb_lo", [N_LO_SUB * nlo_sub_rows, d_lo], F32,
                            kind="ExternalInput")
    # W_high^T stored chunked: w_hi[i, c, j] = W_high.T[c*128 + i, j]
    n_hi_chunks = d_hi // P
    w_hi = nc.dram_tensor("w_hi", [P, n_hi_chunks, d_out], F32, kind="ExternalInput")
    w_lo = nc.dram_tensor("w_lo", [d_lo, d_out], F32, kind="ExternalInput")
    ident = nc.dram_tensor("ident", [P, P], F32, kind="ExternalInput")
    out = nc.dram_tensor(
        "out", [(hi_tiles + lo_tiles) * P, d_out], F32, kind="ExternalOutput"
    )

    from contextlib import ExitStack

    with tile.TileContext(nc) as tc, ExitStack() as ctx:
        const_pool = ctx.enter_context(tc.tile_pool(name="const", bufs=1))
        ids_pool = ctx.enter_context(tc.tile_pool(name="ids", bufs=1))
        xg_pool = ctx.enter_context(tc.tile_pool(name="xg", bufs=3))
        xt_sb_pool = ctx.enter_context(tc.tile_pool(name="xts", bufs=3))
        out_sb_pool = ctx.enter_context(tc.tile_pool(name="osb", bufs=3))
        xt_ps_pool = ctx.enter_context(tc.tile_pool(name="xtp", bufs=2, space="PSUM"))
        out_ps_pool = ctx.enter_context(tc.tile_pool(name="opp", bufs=3, space="PSUM"))

        # Constants / weights, loaded once.
        ident_sb = const_pool.tile([P, P], F32, tag="ident")
        nc.sync.dma_start(ident_sb[:], ident.ap())
        w_hi_sb = const_pool.tile([P, n_hi_chunks, d_out], F32, tag="w_hi")
        nc.sync.dma_start(w_hi_sb[:], w_hi.ap())
        w_lo_sb = const_pool.tile([d_lo, d_out], F32, tag="w_lo")
        nc.sync.dma_start(w_lo_sb[:], w_lo.ap())
        ids_hi_sb = ids_pool.tile([P, hi_tiles * 8], I16, tag="ids_hi")
        if hi_tiles:
            nc.sync.dma_start(ids_hi_sb[:], ids_hi.ap())
        ids_lo_sb = ids_pool.tile([P, lo_tiles * 8], I16, tag="ids_lo")
        if lo_tiles:
            nc.sync.dma_start(ids_lo_sb[:], ids_lo.ap())

        out_ap_full = out.ap()

        def do_span(ids_sb, ids_col0, emb_ap, d_in, n_tiles, row_base, gsub):
            """Process n_tiles 128-token sub-tiles gathered from emb_ap.

            ids_col0: column offset of this span's first token in ids_sb.
            row_base: first output row of this span.
            """
            n_groups = math.ceil(n_tiles / gsub)
            for g in range(n_groups):
                sub0 = g * gsub
                nsub = min(gsub, n_tiles - sub0)
                ntok = nsub * P
                xg = xg_pool.tile([P, nsub, d_in], F32, tag="xg")
                idx_ap = ids_sb[:, ids_col0 + sub0 * 8:
                                ids_col0 + (sub0 + nsub) * 8]
                nc.gpsimd.dma_gather(xg[:], emb_ap, idx_ap, ntok, ntok, d_in,
                                     single_packet=False)
                for q0 in range(0, nsub, QUAD):
                    nq = min(QUAD, nsub - q0)
                    if d_in <= P:
                        # single K-chunk (low expert: K = 64)
                        xt_ps = xt_ps_pool.tile([d_in, nq, P], F32, tag="xtp")
                        for s in range(nq):
                            nc.tensor.transpose(
                                xt_ps[:, s, :], xg[:, q0 + s, :], ident_sb[:]
                            )
                        xt_sb = xt_sb_pool.tile([d_in, nq, P], F32, tag="xts")
                        nc.vector.tensor_copy(xt_sb[:], xt_ps[:])
                        out_ps = out_ps_pool.tile([P, nq, d_out], F32, tag="opp")
                        for s in range(nq):
                            nc.tensor.matmul(
                                out_ps[:, s, :],
                                lhsT=xt_sb[:, s, :],
                                rhs=w_lo_sb[:],
                                start=True,
                                stop=True,
                                skip_group_check=True,
                            )
                    else:
                        # multi K-chunk (high expert: K = 256 -> 2 chunks)
                        nch = d_in // P
                        xt_ps = xt_ps_pool.tile([P, nch * nq, P], F32, tag="xtp")
                        for s in range(nq):
                            for c in range(nch):
                                nc.tensor.transpose(
                                    xt_ps[:, nch * s + c, :],
                                    xg[:, q0 + s, c * P:(c + 1) * P],
                                    ident_sb[:],
                                )
                        xt_sb = xt_sb_pool.tile([P, nch * nq, P], F32, tag="xts")
                        nc.vector.tensor_copy(xt_sb[:], xt_ps[:])
                        out_ps = out_ps_pool.tile([P, nq, d_out], F32, tag="opp")
                        for s in range(nq):
                            for c in range(nch):
                                nc.tensor.matmul(
                                    out_ps[:, s, :],
                                    lhsT=xt_sb[:, nch * s + c, :],
                                    rhs=w_hi_sb[:, c, :],
                                    start=(c == 0),
                                    stop=(c == nch - 1),
                                    skip_group_check=True,
                                )
                    out_sb = out_sb_pool.tile([P, nq, d_out], F32, tag="osb")
                    nc.scalar.copy(out_sb[:], out_ps[:])
                    row0 = row_base + (sub0 + q0) * P
                    dst = out_ap_full[row0:row0 + nq * P, :].rearrange(
                        "(k p) f -> p k f", p=P
                    )
                    nc.sync.dma_start(dst, out_sb[:])

        def body():
            do_span(ids_hi_sb, 0, emb_hi.ap(), d_hi, hi_tiles, 0, HI_GSUB)
            tile_c = 0
            for s in range(N_LO_SUB):
                st = lo_sub_tiles[s]
                if st == 0:
                    continue
                emb_view = emb_lo.ap()[s * nlo_sub_rows:(s + 1) * nlo_sub_rows, :]
                do_span(ids_lo_sb, tile_c * 8, emb_view, d_lo, st,
                        (hi_tiles + tile_c) * P, LO_GSUB)
                tile_c += st

        if n_reps == 1:
            body()
        else:
            with tc.For_i(0, n_reps, 1):
                body()

    nc.compile()
    return nc


_PROGRAM_CACHE = {}


def _get_program(hi_tiles, lo_sub_tiles, n_reps=1):
    key = (hi_tiles, tuple(lo_sub_tiles), n_reps)
    if key not in _PROGRAM_CACHE:
        _PROGRAM_CACHE[key] = _build_program(
            hi_tiles, lo_sub_tiles, NHI_SHARD, NLO_SUBSHARD,
            D_HIGH, D_LOW, D_OUT, n_reps=n_reps,
        )
    return _PROGRAM_CACHE[key]


def _round_up(x, m):
    return ((x + m - 1) // m) * m


def _layout_ids16(sections, total_tokens):
    """Concatenate per-section local int16 ids (each pre-padded), then lay out
    as [128, total/16]: token j at [j % 16, j // 16], tiled 8x down partitions.
    """
    a = np.zeros(total_tokens, np.int16)
    pos = 0
    for sec in sections:
        a[pos:pos + len(sec)] = sec
        pos += len(sec)
    assert pos <= total_tokens
    m = a.reshape(total_tokens // 16, 16).T
    return np.ascontiguousarray(np.tile(m, (8, 1)))


def _route(node_ids):
    """Host-side routing: bucket tokens by owning core / expert / sub-shard.

    Returns (ids64, hi_pos, lo_pos, hi_tiles, lo_sub_tiles) where
    hi_pos[c] is the position array of core c's high tokens and
    lo_pos[c][s] the positions of core c's low-sub-shard-s tokens.
    """
    ids64 = np.asarray(node_ids).astype(np.int64)
    is_hi = ids64 < NUM_HIGH
    core_of = np.where(is_hi, ids64 // NHI_SHARD, (ids64 - NUM_HIGH) // NLO_SHARD)
    hi_pos, lo_pos = [], []
    for c in range(N_CORES):
        sel = np.flatnonzero(core_of == c)
        sel_hi = sel[is_hi[sel]]
        sel_lo = sel[~is_hi[sel]]
        sub = (ids64[sel_lo] - NUM_HIGH - c * NLO_SHARD) // NLO_SUBSHARD
        hi_pos.append(sel_hi)
        lo_pos.append([sel_lo[sub == s] for s in range(N_LO_SUB)])
    hi_cap = _round_up(max(1, max(len(s) for s in hi_pos)), P)
    lo_caps = [
        _round_up(max(len(lo_pos[c][s]) for c in range(N_CORES)), P)
        for s in range(N_LO_SUB)
    ]
    return ids64, hi_pos, lo_pos, hi_cap // P, tuple(x // P for x in lo_caps)


def _make_in_maps(ids64, hi_pos, lo_pos, hi_tiles, lo_sub_tiles,
                  emb_high_w, emb_low_w, W_high, W_low):
    emb_high_w = np.ascontiguousarray(np.asarray(emb_high_w, dtype=np.float32))
    emb_low_w = np.ascontiguousarray(np.asarray(emb_low_w, dtype=np.float32))
    w_hi_host = np.ascontiguousarray(
        np.asarray(W_high, np.float32).T.reshape(D_HIGH // P, P, D_OUT)
        .transpose(1, 0, 2)
    )
    w_lo_host = np.ascontiguousarray(np.asarray(W_low, np.float32).T)
    ident = np.eye(P, dtype=np.float32)

    in_maps = []
    for c in range(N_CORES):
        hi_local = (ids64[hi_pos[c]] - c * NHI_SHARD).astype(np.int16)
        lo_secs = []
        for s in range(N_LO_SUB):
            base = NUM_HIGH + c * NLO_SHARD + s * NLO_SUBSHARD
            sec = (ids64[lo_pos[c][s]] - base).astype(np.int16)
            lo_secs.append(np.pad(sec, (0, lo_sub_tiles[s] * P - len(sec))))
        in_maps.append({
            "ids_hi": _layout_ids16([hi_local], hi_tiles * P),
            "ids_lo": _layout_ids16(lo_secs, sum(lo_sub_tiles) * P),
            "emb_hi": emb_high_w[c * NHI_SHARD:(c + 1) * NHI_SHARD],
            "emb_lo": emb_low_w[c * NLO_SHARD:(c + 1) * NLO_SHARD],
            "w_hi": w_hi_host,
            "w_lo": w_lo_host,
            "ident": ident,
        })
    return in_maps


def _unshard(results, hi_pos, lo_pos, hi_tiles, lo_sub_tiles, batch,
             b_high, b_low):
    out = np.empty((batch, D_OUT), np.float32)
    for c in range(N_CORES):
        r = results[c]["out"]
        if len(hi_pos[c]):
            out[hi_pos[c]] = r[:len(hi_pos[c])]
        row = hi_tiles * P
        for s in range(N_LO_SUB):
            pos = lo_pos[c][s]
            if len(pos):
                out[pos] = r[row:row + len(pos)]
            row += lo_sub_tiles[s] * P
    b_high = np.asarray(b_high, np.float32)
    b_low = np.asarray(b_low, np.float32)
    if b_high.any():
        for c in range(N_CORES):
            out[hi_pos[c]] += b_high
    if b_low.any():
        for c in range(N_CORES):
            for s in range(N_LO_SUB):
                out[lo_pos[c][s]] += b_low
    return out


def kernel(node_ids, emb_high_w, emb_low_w, W_high, b_high, W_low, b_low):
    ids64, hi_pos, lo_pos, hi_tiles, lo_sub_tiles = _route(node_ids)
    nc = _get_program(hi_tiles, lo_sub_tiles)
    in_maps = _make_in_maps(ids64, hi_pos, lo_pos, hi_tiles, lo_sub_tiles,
                            emb_high_w, emb_low_w, W_high, W_low)
    res = run_bass_kernel_spmd(nc, in_maps, core_ids=list(range(N_CORES)))
    return _unshard(res.results, hi_pos, lo_pos, hi_tiles, lo_sub_tiles,
                    len(np.asarray(node_ids)), b_high, b_low)


# revision 12
# speedup vs baseline: 2.0323x; 2.0323x over previous
"""Trainium2 Bass kernel for nn_DynamicEmbedder (MoE-style routed embedding + projection).

Reference computation (fp32):
    is_high = node_ids < 100_000
    out[b]  = is_high ? emb_high_w[id] @ W_high.T + b_high
                      : emb_low_w[id - 100_000] @ W_low.T + b_low

Strategy (8 NeuronCores):
  * Host-side routing ("all-to-all, expert-parallel style" per the sharding
    hint): tokens are bucketed by the core that owns their embedding row.
    Tables are row-sharded 8 ways (high: 12500 rows/core, low: 112500).
    The low shard is further split into 4 sub-shards of 28125 rows so local
    row indices fit the int16 index format of the SWDGE dma_gather op.
  * Duplicate node_ids are deduplicated per core (the device computes one
    projection per distinct row; the host expands back per token). This cuts
    ~20% of all device work -- the SWDGE descriptor generation rate on the
    two GpSimd DGE cores is the kernel's bottleneck, and it scales with the
    number of gathered rows.
  * Device program per 128-row sub-tile:
       dma_gather rows         -> [128 tok, D] in SBUF   (SWDGE, 4 queues)
       PE transpose            -> [D, 128 tok] in PSUM   (low: 2 tiles/op)
       DVE copy                -> SBUF
       PE matmul  X^T as lhsT  -> PSUM [128 tok, 128 feat]  (fp32)
       ACT copy                -> SBUF
       HWDGE DMA               -> DRAM output rows
  * Host inverse-maps the concatenated per-core outputs and adds the
    (normally zero) biases.
"""

import math
import os
import sys

import numpy as np

for _p in ("/opt/trn_rl_repo", "/opt/pypackages"):
    if _p not in sys.path:
        sys.path.append(_p)

import concourse.bass as bass
import concourse.mybir as mybir
import concourse.tile as tile
from concourse import bacc
from concourse.bass_utils import run_bass_kernel_spmd

# Problem constants (hardcoded per the harness contract).
NUM_NODES = 1_000_000
NUM_HIGH = 100_000
NUM_LOW = NUM_NODES - NUM_HIGH
D_HIGH, D_LOW, D_OUT = 256, 64, 128
BATCH = 500_000
N_CORES = 8
NHI_SHARD = NUM_HIGH // N_CORES   # 12500 high rows per core (< int16 max)
NLO_SHARD = NUM_LOW // N_CORES    # 112500 low rows per core
N_LO_SUB = 4
NLO_SUBSHARD = NLO_SHARD // N_LO_SUB  # 28125 rows per low sub-shard (< int16 max)

P = 128                  # SBUF partitions / rows per sub-tile
QUAD = 4                 # sub-tiles per PSUM-output tile (512 rows)
HI_GSUB = 8              # sub-tiles per high gather group (1024 rows)
LO_GSUB = 16             # sub-tiles per low gather group  (2048 rows)
N_QUEUES = 4             # SWDGE queues (desc-gen parallelism caps at ~2x)

F32 = mybir.dt.float32
F32R = mybir.dt.float32r
I16 = mybir.dt.int16


def _build_program(hi_tiles, lo_sub_tiles, nhi_rows, nlo_sub_rows,
                   d_hi, d_lo, d_out, n_reps=1, enable_asserts=False):
    """Build the (single-core, SPMD-replicated) Bass program.

    hi_tiles: number of 128-row sub-tiles in the high section.
    lo_sub_tiles: tuple of sub-tile counts, one per low sub-shard.
    """
    assert d_hi % P == 0 and d_out == P and 2 * d_lo <= P
    assert hi_tiles % QUAD == 0 and all(t % QUAD == 0 for t in lo_sub_tiles)
    lo_sub_tiles = tuple(lo_sub_tiles)
    lo_tiles = sum(lo_sub_tiles)

    nc = bacc.Bacc(
        "TRN2",
        target_bir_lowering=False,
        debug=False,
        enable_asserts=enable_asserts,
        num_devices=N_CORES,
        num_swdge_queues=N_QUEUES,
    )

    # int16 index tensors, [128, n_rows/16]: row j of a section lives at
    # [j % 16, j // 16], replicated 8x down the partition dim (one copy per
    # GpSimd Q7 core).
    ids_hi = nc.dram_tensor("ids_hi", [P, hi_tiles * 8], I16, kind="ExternalInput")
    ids_lo = nc.dram_tensor("ids_lo", [P, lo_tiles * 8], I16, kind="ExternalInput")
    emb_hi = nc.dram_tensor("emb_hi", [nhi_rows, d_hi], F32, kind="ExternalInput")
    emb_lo = nc.dram_tensor("emb_lo", [N_LO_SUB * nlo_sub_rows, d_lo], F32,
                            kind="ExternalInput")
    # W_high^T stored chunked: w_hi[i, c, j] = W_high.T[c*128 + i, j]
    n_hi_chunks = d_hi // P
    w_hi = nc.dram_tensor("w_hi", [P, n_hi_chunks, d_out], F32, kind="ExternalInput")
    # Block-diagonal W_low^T: [128, 2*128] with rows 0:64 = [W_low^T | 0]
    # and rows 64:128 = [0 | W_low^T]. One K=128 matmul against a pair of
    # transposed 64-wide sub-tiles then yields both sub-tiles' projections
    # side by side (N=256), which also unlocks the full-rate fp32r path.
    w_lo = nc.dram_tensor("w_lo", [P, 2 * d_out], F32, kind="ExternalInput")
    ident = nc.dram_tensor("ident", [P, P], F32, kind="ExternalInput")
    out = nc.dram_tensor(
        "out", [(hi_tiles + lo_tiles) * P, d_out], F32, kind="ExternalOutput"
    )

    from contextlib import ExitStack

    qctr = [0]

    with tile.TileContext(nc) as tc, ExitStack() as ctx:
        const_pool = ctx.enter_context(tc.tile_pool(name="const", bufs=1))
        ids_pool = ctx.enter_context(tc.tile_pool(name="ids", bufs=1))
        xg_pool = ctx.enter_context(tc.tile_pool(name="xg", bufs=3))
        xt_sb_pool = ctx.enter_context(tc.tile_pool(name="xts", bufs=3))
        out_sb_pool = ctx.enter_context(tc.tile_pool(name="osb", bufs=3))
        xt_ps_pool = ctx.enter_context(tc.tile_pool(name="xtp", bufs=2, space="PSUM"))
        out_ps_pool = ctx.enter_context(tc.tile_pool(name="opp", bufs=3, space="PSUM"))

        # Constants / weights, loaded once.
        ident_sb = const_pool.tile([P, P], F32, tag="ident")
        nc.sync.dma_start(ident_sb[:], ident.ap())
        w_hi_sb = const_pool.tile([P, n_hi_chunks, d_out], F32, tag="w_hi")
        nc.sync.dma_start(w_hi_sb[:], w_hi.ap())
        w_lo_sb = const_pool.tile([P, 2 * d_out], F32, tag="w_lo")
        nc.sync.dma_start(w_lo_sb[:], w_lo.ap())
        ids_hi_sb = ids_pool.tile([P, hi_tiles * 8], I16, tag="ids_hi")
        if hi_tiles:
            nc.sync.dma_start(ids_hi_sb[:], ids_hi.ap())
        ids_lo_sb = ids_pool.tile([P, lo_tiles * 8], I16, tag="ids_lo")
        if lo_tiles:
            nc.sync.dma_start(ids_lo_sb[:], ids_lo.ap())

        out_ap_full = out.ap()

        def do_span(ids_sb, ids_col0, emb_ap, d_in, n_tiles, row_base, gsub):
            """Process n_tiles 128-row sub-tiles gathered from emb_ap."""
            for g in range(math.ceil(n_tiles / gsub)):
                sub0 = g * gsub
                nsub = min(gsub, n_tiles - sub0)
                ntok = nsub * P
                xg = xg_pool.tile([P, nsub, d_in], F32, tag="xg")
                idx_ap = ids_sb[:, ids_col0 + sub0 * 8:
                                ids_col0 + (sub0 + nsub) * 8]
                nc.gpsimd.dma_gather(xg[:], emb_ap, idx_ap, ntok, ntok, d_in,
                                     single_packet=False,
                                     queue_num=qctr[0] % N_QUEUES)
                qctr[0] += 1
                for q0 in range(0, nsub, QUAD):
                    nq = min(QUAD, nsub - q0)
                    if d_in <= P // 2:
                        # low expert: transpose TWO sub-tiles per PE op
                        npair = nq // 2
                        xt_ps = xt_ps_pool.tile([P, npair, P], F32, tag="xtp")
                        for h in range(npair):
                            s0 = q0 + 2 * h
                            nc.tensor.transpose(
                                xt_ps[:, h, :],
                                xg[:, s0:s0 + 2, :],
                                ident_sb[:],
                            )
                        xt_sb = xt_sb_pool.tile([P, npair, P], F32, tag="xts")
                        nc.vector.tensor_copy(xt_sb[:], xt_ps[:])
                        out_ps = out_ps_pool.tile([P, nq, d_out], F32, tag="opp")
                        for h in range(npair):
                            nc.tensor.matmul(
                                out_ps[:, 2 * h:2 * h + 2, :],
                                lhsT=xt_sb[:, h, :],
                                rhs=w_lo_sb[:],
                                start=True,
                                stop=True,
                                skip_group_check=True,
                            )
                    else:
                        # high expert: K = 256 -> 2 chunks, accumulate
                        nch = d_in // P
                        xt_ps = xt_ps_pool.tile([P, nch * nq, P], F32, tag="xtp")
                        for s in range(nq):
                            for c in range(nch):
                                nc.tensor.transpose(
                                    xt_ps[:, nch * s + c, :],
                                    xg[:, q0 + s, c * P:(c + 1) * P],
                                    ident_sb[:],
                                )
                        xt_sb = xt_sb_pool.tile([P, nch * nq, P], F32, tag="xts")
                        nc.vector.tensor_copy(xt_sb[:], xt_ps[:])
                        out_ps = out_ps_pool.tile([P, nq, d_out], F32, tag="opp")
                        for s in range(nq):
                            for c in range(nch):
                                nc.tensor.matmul(
                                    out_ps[:, s, :],
                                    lhsT=xt_sb[:, nch * s + c, :],
                                    rhs=w_hi_sb[:, c, :],
                                    start=(c == 0),
                                    stop=(c == nch - 1),
                                    skip_group_check=True,
                                )
                    out_sb = out_sb_pool.tile([P, nq, d_out], F32, tag="osb")
                    nc.scalar.copy(out_sb[:], out_ps[:])
                    row0 = row_base + (sub0 + q0) * P
                    dst = out_ap_full[row0:row0 + nq * P, :].rearrange(
                        "(k p) f -> p k f", p=P
                    )
                    nc.sync.dma_start(dst, out_sb[:])

        def body():
            do_span(ids_hi_sb, 0, emb_hi.ap(), d_hi, hi_tiles, 0, HI_GSUB)
            tile_c = 0
            for s in range(N_LO_SUB):
                st = lo_sub_tiles[s]
                if st == 0:
                    continue
                emb_view = emb_lo.ap()[s * nlo_sub_rows:(s + 1) * nlo_sub_rows, :]
                do_span(ids_lo_sb, tile_c * 8, emb_view, d_lo, st,
                        (hi_tiles + tile_c) * P, LO_GSUB)
                tile_c += st

        if n_reps == 1:
            body()
        else:
            with tc.For_i(0, n_reps, 1):
                body()

    nc.compile()
    return nc


_PROGRAM_CACHE = {}


def _get_program(hi_tiles, lo_sub_tiles, n_reps=1):
    key = (hi_tiles, tuple(lo_sub_tiles), n_reps)
    if key not in _PROGRAM_CACHE:
        _PROGRAM_CACHE[key] = _build_program(
            hi_tiles, lo_sub_tiles, NHI_SHARD, NLO_SUBSHARD,
            D_HIGH, D_LOW, D_OUT, n_reps=n_reps,
        )
    return _PROGRAM_CACHE[key]


def _round_up(x, m):
    return ((x + m - 1) // m) * m


def _layout_ids16(sections, total_rows):
    """Concatenate per-section local int16 ids (each pre-padded), then lay out
    as [128, total/16]: row j at [j % 16, j // 16], tiled 8x down partitions.
    """
    a = np.zeros(total_rows, np.int16)
    pos = 0
    for sec in sections:
        a[pos:pos + len(sec)] = sec
        pos += len(sec)
    assert pos <= total_rows
    m = a.reshape(total_rows // 16, 16).T
    return np.ascontiguousarray(np.tile(m, (8, 1)))


def _route(node_ids):
    """Host-side routing: bucket tokens by owning core / expert / sub-shard,
    then deduplicate row ids per bucket.

    Returns (hi, lo, hi_tiles, lo_sub_tiles) where
      hi[c] = (uniq_local_rows, positions, inverse)
      lo[c][s] = (uniq_local_rows, positions, inverse)
    positions index into the batch; inverse maps each position to its row's
    slot in uniq_local_rows.
    """
    ids64 = np.asarray(node_ids).astype(np.int64)
    is_hi = ids64 < NUM_HIGH
    core_of = np.where(is_hi, ids64 // NHI_SHARD, (ids64 - NUM_HIGH) // NLO_SHARD)
    hi, lo = [], []
    for c in range(N_CORES):
        sel = np.flatnonzero(core_of == c)
        sel_hi = sel[is_hi[sel]]
        uniq, inv = np.unique(ids64[sel_hi] - c * NHI_SHARD, return_inverse=True)
        hi.append((uniq.astype(np.int16), sel_hi, inv))
        sel_lo = sel[~is_hi[sel]]
        loc = ids64[sel_lo] - NUM_HIGH - c * NLO_SHARD
        sub = loc // NLO_SUBSHARD
        subs = []
        for s in range(N_LO_SUB):
            m = sub == s
            uniq, inv = np.unique(loc[m] - s * NLO_SUBSHARD, return_inverse=True)
            subs.append((uniq.astype(np.int16), sel_lo[m], inv))
        lo.append(subs)
    hi_cap = _round_up(max(1, max(len(h[0]) for h in hi)), QUAD * P)
    lo_caps = [
        _round_up(max(1, max(lo[c][s][0].size for c in range(N_CORES))), QUAD * P)
        for s in range(N_LO_SUB)
    ]
    return hi, lo, hi_cap // P, tuple(x // P for x in lo_caps)


def _make_in_maps(hi, lo, hi_tiles, lo_sub_tiles,
                  emb_high_w, emb_low_w, W_high, W_low):
    emb_high_w = np.ascontiguousarray(np.asarray(emb_high_w, dtype=np.float32))
    emb_low_w = np.ascontiguousarray(np.asarray(emb_low_w, dtype=np.float32))
    w_hi_host = np.ascontiguousarray(
        np.asarray(W_high, np.float32).T.reshape(D_HIGH // P, P, D_OUT)
        .transpose(1, 0, 2)
    )
    w_lo_t = np.asarray(W_low, np.float32).T
    z64 = np.zeros((D_LOW, D_OUT), np.float32)
    w_lo_host = np.ascontiguousarray(np.block([[w_lo_t, z64], [z64, w_lo_t]]))
    ident = np.eye(P, dtype=np.float32)

    in_maps = []
    for c in range(N_CORES):
        lo_secs = [np.pad(lo[c][s][0], (0, lo_sub_tiles[s] * P - len(lo[c][s][0])))
                   for s in range(N_LO_SUB)]
        in_maps.append({
            "ids_hi": _layout_ids16([hi[c][0]], hi_tiles * P),
            "ids_lo": _layout_ids16(lo_secs, sum(lo_sub_tiles) * P),
            "emb_hi": emb_high_w[c * NHI_SHARD:(c + 1) * NHI_SHARD],
            "emb_lo": emb_low_w[c * NLO_SHARD:(c + 1) * NLO_SHARD],
            "w_hi": w_hi_host,
            "w_lo": w_lo_host,
            "ident": ident,
        })
    return in_maps


def _unshard(results, hi, lo, hi_tiles, lo_sub_tiles, batch, b_high, b_low):
    out = np.empty((batch, D_OUT), np.float32)
    for c in range(N_CORES):
        r = results[c]["out"]
        uniq, pos, inv = hi[c]
        if len(pos):
            out[pos] = r[inv]
        row = hi_tiles * P
        for s in range(N_LO_SUB):
            uniq, pos, inv = lo[c][s]
            if len(pos):
                out[pos] = r[row + inv]
            row += lo_sub_tiles[s] * P
    b_high = np.asarray(b_high, np.float32)
    b_low = np.asarray(b_low, np.float32)
    if b_high.any():
        for c in range(N_CORES):
            out[hi[c][1]] += b_high
    if b_low.any():
        for c in range(N_CORES):
            for s in range(N_LO_SUB):
                out[lo[c][s][1]] += b_low
    return out


def kernel(node_ids, emb_high_w, emb_low_w, W_high, b_high, W_low, b_low):
    hi, lo, hi_tiles, lo_sub_tiles = _route(node_ids)
    nc = _get_program(hi_tiles, lo_sub_tiles)
    in_maps = _make_in_maps(hi, lo, hi_tiles, lo_sub_tiles,
                            emb_high_w, emb_low_w, W_high, W_low)
    res = run_bass_kernel_spmd(nc, in_maps, core_ids=list(range(N_CORES)))
    return _unshard(res.results, hi, lo, hi_tiles, lo_sub_tiles,
                    len(np.asarray(node_ids)), b_high, b_low)


# revision 14
# speedup vs baseline: 2.7844x; 1.3701x over previous
"""Trainium2 Bass kernel for nn_DynamicEmbedder (MoE-style routed embedding + projection).

Reference computation (fp32):
    is_high = node_ids < 100_000
    out[b]  = is_high ? emb_high_w[id] @ W_high.T + b_high
                      : emb_low_w[id - 100_000] @ W_low.T + b_low

Strategy (8 NeuronCores):
  * Host-side routing ("all-to-all, expert-parallel style" per the sharding
    hint): tokens are bucketed by the core that owns their embedding row.
    Tables are row-sharded 8 ways (high: 12500 rows/core, low: 112500).
    The low shard is further split into 4 sub-shards of 28125 rows so local
    row indices fit the int16 index format of the SWDGE dma_gather op.
  * The kernel bottleneck is SWDGE descriptor generation on the GpSimd DGE
    cores (one descriptor per gathered row, ~4 ns each with gathers spread
    over 4 SWDGE queues). Two host-side reductions attack it:
      - dedup: the device computes one projection per DISTINCT row; the host
        expands back per token (~20% fewer rows);
      - pair-gather: when rows 2i and 2i+1 are both needed, they are fetched
        with ONE 512-byte descriptor from a paired view of the table
        (~20% fewer descriptors again for the low expert).
  * Device pipeline per 128-slot sub-tile:
       dma_gather rows         -> [128, D] in SBUF      (SWDGE, 4 queues)
       PE transpose            -> [D, 128] in PSUM      (fp32, 2 cyc/row)
       DVE copy                -> SBUF
       PE matmul  X^T as lhsT  -> PSUM [slots, features] (fp32; low expert
            uses a block-diagonal [W_low^T 0; 0 W_low^T] so one K=128
            matmul projects two 64-wide rows at once)
       ACT copy                -> SBUF
       HWDGE DMA               -> DRAM output rows
  * Host inverse-maps the concatenated per-core outputs and adds the
    (normally zero) biases.
"""

import math
import os
import sys

import numpy as np

for _p in ("/opt/trn_rl_repo", "/opt/pypackages"):
    if _p not in sys.path:
        sys.path.append(_p)

import concourse.bass as bass
import concourse.mybir as mybir
import concourse.tile as tile
from concourse import bacc
from concourse.bass_utils import run_bass_kernel_spmd

# Problem constants (hardcoded per the harness contract).
NUM_NODES = 1_000_000
NUM_HIGH = 100_000
NUM_LOW = NUM_NODES - NUM_HIGH
D_HIGH, D_LOW, D_OUT = 256, 64, 128
BATCH = 500_000
N_CORES = 8
NHI_SHARD = NUM_HIGH // N_CORES   # 12500 high rows per core (< int16 max)
NLO_SHARD = NUM_LOW // N_CORES    # 112500 low rows per core
N_LO_SUB = 4
NLO_SUBSHARD = NLO_SHARD // N_LO_SUB  # 28125 rows per low sub-shard (< int16 max)
N_PAIRS = NLO_SUBSHARD // 2           # aligned row pairs per low sub-shard

P = 128                  # SBUF partitions / rows per sub-tile
QUAD = 4                 # single sub-tiles per PSUM-output tile (512 rows)
PAIR_QUAD = 2            # pair sub-tiles per PSUM-output tile (512 rows)
HI_GSUB = 8              # sub-tiles per high gather group (1 KiB rows)
LO_GSUB = 16             # sub-tiles per low single gather group (256 B rows)
PAIR_GSUB = 8            # sub-tiles per low pair gather group (512 B elems)
N_QUEUES = 4             # SWDGE queues (desc-gen parallelism caps at ~2x)

F32 = mybir.dt.float32
I16 = mybir.dt.int16


def _build_program(hi_tiles, lo_plans, nhi_rows, nlo_sub_rows,
                   d_hi, d_lo, d_out, n_reps=1, enable_asserts=False,
                   ablate="full"):
    """Build the (single-core, SPMD-replicated) Bass program.

    hi_tiles: number of 128-row sub-tiles in the high section.
    lo_plans: tuple of (pair_tiles, single_tiles) per low sub-shard.
    """
    assert d_hi % P == 0 and d_out == P and 2 * d_lo <= P
    assert hi_tiles % QUAD == 0
    lo_plans = tuple((int(pt), int(st)) for pt, st in lo_plans)
    for pt, st in lo_plans:
        assert pt % PAIR_QUAD == 0 and st % QUAD == 0
    lo_idx_tiles = sum(pt + st for pt, st in lo_plans)
    out_rows = (hi_tiles + sum(2 * pt + st for pt, st in lo_plans)) * P

    nc = bacc.Bacc(
        "TRN2",
        target_bir_lowering=False,
        debug=False,
        enable_asserts=enable_asserts,
        num_devices=N_CORES,
        num_swdge_queues=N_QUEUES,
    )

    # int16 index tensors, [128, n/16]: index j of a section lives at
    # [j % 16, j // 16], replicated 8x down the partition dim (one copy per
    # GpSimd Q7 core).
    ids_hi = nc.dram_tensor("ids_hi", [P, hi_tiles * 8], I16, kind="ExternalInput")
    ids_lo = nc.dram_tensor("ids_lo", [P, lo_idx_tiles * 8], I16,
                            kind="ExternalInput")
    emb_hi = nc.dram_tensor("emb_hi", [nhi_rows, d_hi], F32, kind="ExternalInput")
    emb_lo = nc.dram_tensor("emb_lo", [N_LO_SUB * nlo_sub_rows, d_lo], F32,
                            kind="ExternalInput")
    # W_high^T stored chunked: w_hi[i, c, j] = W_high.T[c*128 + i, j]
    n_hi_chunks = d_hi // P
    w_hi = nc.dram_tensor("w_hi", [P, n_hi_chunks, d_out], F32, kind="ExternalInput")
    # Block-diagonal W_low^T: [128, 2*128] with rows 0:64 = [W_low^T | 0]
    # and rows 64:128 = [0 | W_low^T].
    w_lo = nc.dram_tensor("w_lo", [P, 2 * d_out], F32, kind="ExternalInput")
    ident = nc.dram_tensor("ident", [P, P], F32, kind="ExternalInput")
    out = nc.dram_tensor("out", [out_rows, d_out], F32, kind="ExternalOutput")

    from contextlib import ExitStack

    qctr = [0]
    compute = ablate == "full"

    with tile.TileContext(nc) as tc, ExitStack() as ctx:
        const_pool = ctx.enter_context(tc.tile_pool(name="const", bufs=1))
        ids_pool = ctx.enter_context(tc.tile_pool(name="ids", bufs=1))
        xg_pool = ctx.enter_context(tc.tile_pool(name="xg", bufs=3))
        xt_sb_pool = ctx.enter_context(tc.tile_pool(name="xts", bufs=3))
        out_sb_pool = ctx.enter_context(tc.tile_pool(name="osb", bufs=3))
        xt_ps_pool = ctx.enter_context(tc.tile_pool(name="xtp", bufs=2, space="PSUM"))
        out_ps_pool = ctx.enter_context(tc.tile_pool(name="opp", bufs=3, space="PSUM"))

        # Constants / weights, loaded once.
        ident_sb = const_pool.tile([P, P], F32, tag="ident")
        nc.sync.dma_start(ident_sb[:], ident.ap())
        w_hi_sb = const_pool.tile([P, n_hi_chunks, d_out], F32, tag="w_hi")
        nc.sync.dma_start(w_hi_sb[:], w_hi.ap())
        w_lo_sb = const_pool.tile([P, 2 * d_out], F32, tag="w_lo")
        nc.sync.dma_start(w_lo_sb[:], w_lo.ap())
        ids_hi_sb = ids_pool.tile([P, hi_tiles * 8], I16, tag="ids_hi")
        if hi_tiles:
            nc.sync.dma_start(ids_hi_sb[:], ids_hi.ap())
        ids_lo_sb = ids_pool.tile([P, lo_idx_tiles * 8], I16, tag="ids_lo")
        if lo_idx_tiles:
            nc.sync.dma_start(ids_lo_sb[:], ids_lo.ap())

        out_ap_full = out.ap()

        def gather(ids_sb, ids_col0, emb_ap, d_elem, sub0, nsub):
            xg = xg_pool.tile([P, nsub, d_elem], F32, tag="xg")
            idx_ap = ids_sb[:, ids_col0 + sub0 * 8:ids_col0 + (sub0 + nsub) * 8]
            nc.gpsimd.dma_gather(xg[:], emb_ap, idx_ap, nsub * P, nsub * P,
                                 d_elem, single_packet=False,
                                 queue_num=qctr[0] % N_QUEUES)
            qctr[0] += 1
            return xg

        def emit_out(out_sb, row0, n_rows, shape):
            dst = out_ap_full[row0:row0 + n_rows, :].rearrange(shape, p=P)
            nc.sync.dma_start(dst, out_sb[:])

        def span_hi(ids_col0, n_tiles, row_base):
            for g in range(math.ceil(n_tiles / HI_GSUB)):
                sub0 = g * HI_GSUB
                nsub = min(HI_GSUB, n_tiles - sub0)
                xg = gather(ids_hi_sb, ids_col0, emb_hi.ap(), d_hi, sub0, nsub)
                if not compute:
                    continue
                for q0 in range(0, nsub, QUAD):
                    nq = min(QUAD, nsub - q0)
                    nch = n_hi_chunks
                    xt_ps = xt_ps_pool.tile([P, nch * nq, P], F32, tag="xtp")
                    for s in range(nq):
                        for c in range(nch):
                            nc.tensor.transpose(
                                xt_ps[:, nch * s + c, :],
                                xg[:, q0 + s, c * P:(c + 1) * P], ident_sb[:])
                    xt_sb = xt_sb_pool.tile([P, nch * nq, P], F32, tag="xts")
                    nc.vector.tensor_copy(xt_sb[:], xt_ps[:])
                    out_ps = out_ps_pool.tile([P, nq, d_out], F32, tag="opp")
                    for s in range(nq):
                        for c in range(nch):
                            nc.tensor.matmul(
                                out_ps[:, s, :], lhsT=xt_sb[:, nch * s + c, :],
                                rhs=w_hi_sb[:, c, :], start=(c == 0),
                                stop=(c == nch - 1), skip_group_check=True)
                    out_sb = out_sb_pool.tile([P, nq, d_out], F32, tag="osb")
                    nc.scalar.copy(out_sb[:], out_ps[:])
                    emit_out(out_sb, row_base + (sub0 + q0) * P, nq * P,
                             "(k p) f -> p k f")

        def span_pairs(ids_col0, emb_pair_ap, n_tiles, row_base):
            # each gathered element is a 512 B pair of rows [2i, 2i+1];
            # each sub-tile of 128 pairs yields 256 output rows.
            for g in range(math.ceil(n_tiles / PAIR_GSUB)):
                sub0 = g * PAIR_GSUB
                nsub = min(PAIR_GSUB, n_tiles - sub0)
                xg = gather(ids_lo_sb, ids_col0, emb_pair_ap, 2 * d_lo,
                            sub0, nsub)
                if not compute:
                    continue
                for q0 in range(0, nsub, PAIR_QUAD):
                    nq = min(PAIR_QUAD, nsub - q0)
                    xt_ps = xt_ps_pool.tile([P, nq, P], F32, tag="xtp")
                    for s in range(nq):
                        nc.tensor.transpose(
                            xt_ps[:, s, :], xg[:, q0 + s, :], ident_sb[:])
                    xt_sb = xt_sb_pool.tile([P, nq, P], F32, tag="xts")
                    nc.vector.tensor_copy(xt_sb[:], xt_ps[:])
                    out_ps = out_ps_pool.tile([P, nq, 2, d_out], F32, tag="opp")
                    for s in range(nq):
                        nc.tensor.matmul(
                            out_ps[:, s, :, :], lhsT=xt_sb[:, s, :],
                            rhs=w_lo_sb[:], start=True, stop=True,
                            skip_group_check=True)
                    out_sb = out_sb_pool.tile([P, nq, 2, d_out], F32, tag="osb")
                    nc.scalar.copy(out_sb[:], out_ps[:])
                    dst = out_ap_full[
                        row_base + (sub0 + q0) * 2 * P:
                        row_base + (sub0 + q0) * 2 * P + nq * 2 * P, :
                    ].rearrange("(s r p) f -> p s r f", p=P, r=2)
                    nc.sync.dma_start(dst, out_sb[:])

        def span_singles(ids_col0, emb_ap, n_tiles, row_base):
            for g in range(math.ceil(n_tiles / LO_GSUB)):
                sub0 = g * LO_GSUB
                nsub = min(LO_GSUB, n_tiles - sub0)
                xg = gather(ids_lo_sb, ids_col0, emb_ap, d_lo, sub0, nsub)
                if not compute:
                    continue
                for q0 in range(0, nsub, QUAD):
                    nq = min(QUAD, nsub - q0)
                    npair = nq // 2
                    xt_ps = xt_ps_pool.tile([P, npair, P], F32, tag="xtp")
                    for h in range(npair):
                        s0 = q0 + 2 * h
                        nc.tensor.transpose(
                            xt_ps[:, h, :], xg[:, s0:s0 + 2, :], ident_sb[:])
                    xt_sb = xt_sb_pool.tile([P, npair, P], F32, tag="xts")
                    nc.vector.tensor_copy(xt_sb[:], xt_ps[:])
                    out_ps = out_ps_pool.tile([P, nq, d_out], F32, tag="opp")
                    for h in range(npair):
                        nc.tensor.matmul(
                            out_ps[:, 2 * h:2 * h + 2, :], lhsT=xt_sb[:, h, :],
                            rhs=w_lo_sb[:], start=True, stop=True,
                            skip_group_check=True)
                    out_sb = out_sb_pool.tile([P, nq, d_out], F32, tag="osb")
                    nc.scalar.copy(out_sb[:], out_ps[:])
                    emit_out(out_sb, row_base + (sub0 + q0) * P, nq * P,
                             "(k p) f -> p k f")

        def body():
            span_hi(0, hi_tiles, 0)
            col = 0
            row = hi_tiles * P
            for s in range(N_LO_SUB):
                pt, st = lo_plans[s]
                base = s * nlo_sub_rows
                if pt:
                    pair_ap = emb_lo.ap()[base:base + 2 * (nlo_sub_rows // 2), :] \
                        .rearrange("(n two) d -> n (two d)", two=2)
                    span_pairs(col * 8, pair_ap, pt, row)
                    col += pt
                    row += pt * 2 * P
                if st:
                    sub_ap = emb_lo.ap()[base:base + nlo_sub_rows, :]
                    span_singles(col * 8, sub_ap, st, row)
                    col += st
                    row += st * P

        if n_reps == 1:
            body()
        else:
            with tc.For_i(0, n_reps, 1):
                body()

    nc.compile()
    return nc


_PROGRAM_CACHE = {}


def _get_program(hi_tiles, lo_plans, n_reps=1, ablate="full"):
    key = (hi_tiles, tuple(lo_plans), n_reps, ablate)
    if key not in _PROGRAM_CACHE:
        _PROGRAM_CACHE[key] = _build_program(
            hi_tiles, lo_plans, NHI_SHARD, NLO_SUBSHARD,
            D_HIGH, D_LOW, D_OUT, n_reps=n_reps, ablate=ablate,
        )
    return _PROGRAM_CACHE[key]


def _round_up(x, m):
    return ((x + m - 1) // m) * m


def _layout_ids16(sections, total):
    """Concatenate per-section local int16 ids (each pre-padded), then lay out
    as [128, total/16]: index j at [j % 16, j // 16], tiled 8x down partitions.
    """
    a = np.zeros(total, np.int16)
    pos = 0
    for sec in sections:
        a[pos:pos + len(sec)] = sec
        pos += len(sec)
    assert pos <= total
    m = a.reshape(total // 16, 16).T
    return np.ascontiguousarray(np.tile(m, (8, 1)))


def _route(node_ids):
    """Host-side routing: bucket tokens by owning core / expert / sub-shard,
    dedup row ids, and split low rows into aligned pairs vs singles.

    Returns (hi, lo, hi_tiles, lo_plans):
      hi[c] = (uniq_rows_i16, positions, inverse)
      lo[c][s] = (pair_ids_i16, single_ids_i16, positions, inverse, uniq)
    """
    ids64 = np.asarray(node_ids).astype(np.int64)
    is_hi = ids64 < NUM_HIGH
    core_of = np.where(is_hi, ids64 // NHI_SHARD, (ids64 - NUM_HIGH) // NLO_SHARD)
    hi, lo = [], []
    for c in range(N_CORES):
        sel = np.flatnonzero(core_of == c)
        sel_hi = sel[is_hi[sel]]
        uniq, inv = np.unique(ids64[sel_hi] - c * NHI_SHARD, return_inverse=True)
        hi.append((uniq.astype(np.int16), sel_hi, inv))
        sel_lo = sel[~is_hi[sel]]
        loc = ids64[sel_lo] - NUM_HIGH - c * NLO_SHARD
        sub = loc // NLO_SUBSHARD
        subs = []
        for s in range(N_LO_SUB):
            m = sub == s
            uniq, inv = np.unique(loc[m] - s * NLO_SUBSHARD, return_inverse=True)
            pres = np.zeros(NLO_SUBSHARD + 1, bool)
            pres[uniq] = True
            both = pres[0:2 * N_PAIRS:2] & pres[1:2 * N_PAIRS:2]
            pairs = np.flatnonzero(both)
            in_pair = np.zeros(NLO_SUBSHARD, bool)
            in_pair[2 * pairs] = True
            in_pair[2 * pairs + 1] = True
            singles = uniq[~in_pair[uniq]]
            subs.append((pairs.astype(np.int16), singles.astype(np.int16),
                         sel_lo[m], inv, uniq))
        lo.append(subs)
    hi_tiles = _round_up(max(1, max(len(h[0]) for h in hi)), QUAD * P) // P
    lo_plans = []
    for s in range(N_LO_SUB):
        pt = _round_up(max(lo[c][s][0].size for c in range(N_CORES)),
                       PAIR_QUAD * P) // P
        st = _round_up(max(lo[c][s][1].size for c in range(N_CORES)),
                       QUAD * P) // P
        lo_plans.append((pt, st))
    return hi, lo, hi_tiles, tuple(lo_plans)


def _lo_rowmap(pairs, singles, uniq, pair_base, single_base):
    """Device-local output row for each slot of uniq."""
    rowmap = np.empty(len(uniq), np.int64)
    half = uniq // 2
    isp = np.zeros(len(uniq), bool)
    if len(pairs):
        pi = np.searchsorted(pairs, half)
        pi_c = np.minimum(pi, len(pairs) - 1)
        isp = (pairs[pi_c] == half) & (uniq < 2 * N_PAIRS)
        j = pi_c[isp]
        r = uniq[isp] % 2
        rowmap[isp] = pair_base + (j // P) * 2 * P + r * P + (j % P)
    if len(singles):
        si = np.searchsorted(singles, uniq[~isp])
        rowmap[~isp] = single_base + si
    return rowmap


def _make_in_maps(hi, lo, hi_tiles, lo_plans,
                  emb_high_w, emb_low_w, W_high, W_low):
    emb_high_w = np.ascontiguousarray(np.asarray(emb_high_w, dtype=np.float32))
    emb_low_w = np.ascontiguousarray(np.asarray(emb_low_w, dtype=np.float32))
    w_hi_host = np.ascontiguousarray(
        np.asarray(W_high, np.float32).T.reshape(D_HIGH // P, P, D_OUT)
        .transpose(1, 0, 2)
    )
    w_lo_t = np.asarray(W_low, np.float32).T
    z64 = np.zeros((D_LOW, D_OUT), np.float32)
    w_lo_host = np.ascontiguousarray(np.block([[w_lo_t, z64], [z64, w_lo_t]]))
    ident = np.eye(P, dtype=np.float32)

    in_maps = []
    for c in range(N_CORES):
        lo_secs = []
        for s in range(N_LO_SUB):
            pairs, singles = lo[c][s][0], lo[c][s][1]
            pt, st = lo_plans[s]
            lo_secs.append(np.pad(pairs, (0, pt * P - len(pairs))))
            lo_secs.append(np.pad(singles, (0, st * P - len(singles))))
        in_maps.append({
            "ids_hi": _layout_ids16([hi[c][0]], hi_tiles * P),
            "ids_lo": _layout_ids16(
                lo_secs, sum(pt + st for pt, st in lo_plans) * P),
            "emb_hi": emb_high_w[c * NHI_SHARD:(c + 1) * NHI_SHARD],
            "emb_lo": emb_low_w[c * NLO_SHARD:(c + 1) * NLO_SHARD],
            "w_hi": w_hi_host,
            "w_lo": w_lo_host,
            "ident": ident,
        })
    return in_maps


def _unshard(results, hi, lo, hi_tiles, lo_plans, batch, b_high, b_low):
    out = np.empty((batch, D_OUT), np.float32)
    for c in range(N_CORES):
        r = results[c]["out"]
        uniq, pos, inv = hi[c]
        if len(pos):
            out[pos] = r[inv]
        row = hi_tiles * P
        for s in range(N_LO_SUB):
            pairs, singles, pos, inv, uniq = lo[c][s]
            pt, st = lo_plans[s]
            if len(pos):
                rowmap = _lo_rowmap(pairs, singles, uniq,
                                    row, row + pt * 2 * P)
                out[pos] = r[rowmap[inv]]
            row += pt * 2 * P + st * P
    b_high = np.asarray(b_high, np.float32)
    b_low = np.asarray(b_low, np.float32)
    if b_high.any():
        for c in range(N_CORES):
            out[hi[c][1]] += b_high
    if b_low.any():
        for c in range(N_CORES):
            for s in range(N_LO_SUB):
                out[lo[c][s][2]] += b_low
    return out


def kernel(node_ids, emb_high_w, emb_low_w, W_high, b_high, W_low, b_low):
    hi, lo, hi_tiles, lo_plans = _route(node_ids)
    nc = _get_program(hi_tiles, lo_plans)
    in_maps = _make_in_maps(hi, lo, hi_tiles, lo_plans,
                            emb_high_w, emb_low_w, W_high, W_low)
    res = run_bass_kernel_spmd(nc, in_maps, core_ids=list(range(N_CORES)))
    return _unshard(res.results, hi, lo, hi_tiles, lo_plans,
                    len(np.asarray(node_ids)), b_high, b_low)
